# revision 5
# baseline (speedup 1.0000x reference)
"""Trainium2 Bass kernel: single-head causal attention.

Reference computation (per batch b):
    K = x @ Wk; Q = x @ Wq; V = x @ Wv          (x: [T, C], W*: [C, H])
    S = Q @ K^T * C**-0.5, causal-masked
    out = softmax(S) @ V                        ([T, H])

Shapes hardcoded: B=4, T=4096, C=1024, H=64, f32.

Sharding: 8 cores = 4 batches x 2 causal-balanced query-halves.
  half 0 -> query blocks {0,1,6,7} of 512 rows, half 1 -> {2,3,4,5}.
Both halves have equal attention work (72 tk-tiles of 128) but different
instruction streams, so we compile TWO 4-core SPMD graphs (one per half)
and dispatch them concurrently on devices [0:4] and [4:8].

Per-core dataflow (all in one NeuronCore):
  1. Stream x in 512-row chunks; PE-transpose to xT [c,t] layout.
  2. Project: KT/VT[64,4096], QT[64,2048] = W.T @ xT  (PSUM accum over c).
  3. PE-transpose VT -> V [t, 65] with an appended ones column (row-sum trick).
  4. Per query block: ST[tk,tq] = K Q^T tiles, diag-masked, exp via ACT
     (scale 1/32 folded in), PV accumulated into OT[65, 512] PSUM,
     row 64 = softmax denominator. Transpose OT, normalize, DMA out.
"""

import os
import threading

import numpy as np

B, T, C, H = 4, 4096, 1024, 64
SCALE = float(C) ** -0.5  # 1/32
MASKVAL = -1.0e5  # pre-scale additive mask; exp(-1e5/32) == 0
QBLK = 512  # query block rows
HALF_BLOCKS = {0: (0, 1, 6, 7), 1: (2, 3, 4, 5)}
N_CORES_PER_GRAPH = 4

# matmul input dtype: "f32" (safe, 4 cyc/row) or "f32r" (TF32-ish, 1 cyc/row)
MM_DT_NAME = os.environ.get("BASS_MM_DT", "f32")

_compiled = {}
_compile_lock = threading.Lock()


def _build_variant(half: int):
    import concourse.bass as bass
    import concourse.mybir as mybir
    from concourse import bacc, tile
    from concourse._compat import get_trn_type
    from concourse.masks import make_identity

    f32 = mybir.dt.float32
    mm_dt = {"f32": mybir.dt.float32, "f32r": mybir.dt.float32r}[MM_DT_NAME]
    AF = mybir.ActivationFunctionType
    ALU = mybir.AluOpType

    def mm(out, lhsT, rhs, **kw):
        if mm_dt is not f32:
            lhsT = lhsT.bitcast(mm_dt)
            rhs = rhs.bitcast(mm_dt)
        nc.tensor.matmul(out, lhsT, rhs, **kw)

    nc = bacc.Bacc(
        get_trn_type() or "TRN2",
        target_bir_lowering=False,
        debug=False,
        num_devices=N_CORES_PER_GRAPH,
    )

    x_d = nc.dram_tensor("x", [T, C], f32, kind="ExternalInput").ap()
    wk_d = nc.dram_tensor("Wk", [C, H], f32, kind="ExternalInput").ap()
    wq_d = nc.dram_tensor("Wq", [C, H], f32, kind="ExternalInput").ap()
    wv_d = nc.dram_tensor("Wv", [C, H], f32, kind="ExternalInput").ap()
    out_d = nc.dram_tensor("out", [2048, H], f32, kind="ExternalOutput").ap()

    own = HALF_BLOCKS[half]
    n_chunks = T // QBLK  # 8 chunks of 512 rows
    n_ctiles = C // 128  # 8 contraction tiles

    with tile.TileContext(nc) as tc:
        with (
            tc.tile_pool(name="const", bufs=1) as pconst,
            tc.tile_pool(name="persist", bufs=1) as ppersist,
            tc.tile_pool(name="xin", bufs=2) as pxin,
            tc.tile_pool(name="xt", bufs=2) as pxt,
            tc.tile_pool(name="pt", bufs=3) as ppt,
            tc.tile_pool(name="otsb", bufs=2) as potsb,
            tc.tile_pool(name="osmall", bufs=3) as posmall,
            tc.tile_pool(name="ps_big", bufs=2, space="PSUM") as pbig,
            tc.tile_pool(name="ps_proj", bufs=2, space="PSUM") as pproj,
            tc.tile_pool(name="ps_acc", bufs=2, space="PSUM") as pacc,
            tc.tile_pool(name="ps_small", bufs=2, space="PSUM") as psmall,
        ):
            # ---- constants ----
            ident = pconst.tile([128, 128], f32, tag="ident")
            make_identity(nc, ident[:])
            # big_mask[p, j] = MASKVAL if p > j - 512 else 0   (j in [0,640))
            # diag tile with offset d uses columns [512-d, 640)
            big_mask = pconst.tile([128, 640], f32, tag="mask")
            nc.gpsimd.memset(big_mask[:], 0.0)
            nc.gpsimd.affine_select(
                out=big_mask[:],
                in_=big_mask[:],
                compare_op=ALU.is_ge,
                fill=MASKVAL,
                base=-512,
                channel_multiplier=-1,
                pattern=[[1, 640]],
            )

            # ---- weights: [128, 8, 64] (c_in-partition, c_tile, h) ----
            w_sb = {}
            for name, wd in (("k", wk_d), ("q", wq_d), ("v", wv_d)):
                w = pconst.tile([128, n_ctiles, H], f32, tag=f"w{name}")
                for cb in range(n_ctiles):
                    nc.sync.dma_start(w[:, cb, :], wd[128 * cb : 128 * (cb + 1), :])
                w_sb[name] = w

            # ---- persistent projections ----
            kt_all = ppersist.tile([H, T], f32, tag="kt")
            vt_all = ppersist.tile([H, T], f32, tag="vt")
            qt_all = ppersist.tile([H, 2048], f32, tag="qt")
            v_sb = ppersist.tile([128, T // 128, H + 1], f32, tag="v")
            nc.vector.memset(v_sb[:, :, H : H + 1], 1.0)

            # ---- projection: stream x, transpose, project ----
            own_set = set(own)
            qi_of_chunk = {blk: i for i, blk in enumerate(own)}
            for ci in range(n_chunks):
                t0 = QBLK * ci
                x_chunk = pxin.tile([128, 4, C], f32, tag="xc")
                nc.sync.dma_start(
                    x_chunk[:],
                    x_d[t0 : t0 + QBLK, :].rearrange("(a p) c -> p a c", p=128),
                )
                xt_sb = pxt.tile([128, n_ctiles, QBLK], f32, tag="xt")
                for cb in range(n_ctiles):
                    xt_ps = pbig.tile([128, QBLK], f32, tag="big")
                    for a in range(4):
                        nc.tensor.transpose(
                            xt_ps[:, 128 * a : 128 * (a + 1)],
                            x_chunk[:, a, 128 * cb : 128 * (cb + 1)],
                            ident[:],
                        )
                    nc.scalar.copy(xt_sb[:, cb, :], xt_ps[:])

                for name, dst, dslice in (
                    ("k", kt_all, (t0, t0 + QBLK)),
                    ("v", vt_all, (t0, t0 + QBLK)),
                    (
                        "q",
                        qt_all,
                        None
                        if ci not in own_set
                        else (QBLK * qi_of_chunk[ci], QBLK * (qi_of_chunk[ci] + 1)),
                    ),
                ):
                    if dslice is None:
                        continue
                    prj = pproj.tile([H, QBLK], f32, tag="proj")
                    for cb in range(n_ctiles):
                        mm(
                            prj[:],
                            w_sb[name][:, cb, :],
                            xt_sb[:, cb, :],
                            start=(cb == 0),
                            stop=(cb == n_ctiles - 1),
                        )
                    nc.scalar.copy(dst[:, dslice[0] : dslice[1]], prj[:])

            # ---- V: [64, T] -> [t, 65] blocks (append ones col) ----
            for tt in range(T // 128):
                vps = psmall.tile([128, H], f32, tag="small")
                nc.tensor.transpose(
                    vps[:], vt_all[:, 128 * tt : 128 * (tt + 1)], ident[0:H, 0:H]
                )
                nc.scalar.copy(v_sb[:, tt, 0:H], vps[:])

            # ---- attention per query block ----
            for qb, blk in enumerate(own):
                g_q = QBLK * blk
                n_tk = (g_q + QBLK) // 128
                ot_ps = pacc.tile([H + 1, QBLK], f32, tag="acc")
                for tk in range(n_tk):
                    st_ps = pbig.tile([128, QBLK], f32, tag="big")
                    mm(
                        st_ps[:],
                        kt_all[:, 128 * tk : 128 * (tk + 1)],
                        qt_all[:, QBLK * qb : QBLK * (qb + 1)],
                        start=True,
                        stop=True,
                    )
                    d = 128 * tk - g_q
                    if d >= 0:
                        nc.vector.tensor_tensor(
                            st_ps[:, 0 : d + 128],
                            st_ps[:, 0 : d + 128],
                            big_mask[:, 512 - d : 640],
                            op=ALU.add,
                        )
                    pt_sb = ppt.tile([128, QBLK], f32, tag="pt")
                    nc.scalar.activation(pt_sb[:], st_ps[:], AF.Exp, scale=SCALE)
                    mm(
                        ot_ps[:],
                        v_sb[:, tk, :],
                        pt_sb[:],
                        start=(tk == 0),
                        stop=(tk == n_tk - 1),
                    )
                ot_sb = potsb.tile([H + 1, QBLK], f32, tag="ot")
                nc.scalar.copy(ot_sb[:], ot_ps[:])
                for a in range(4):
                    o_ps = psmall.tile([128, H + 1], f32, tag="small")
                    nc.tensor.transpose(
                        o_ps[:],
                        ot_sb[:, 128 * a : 128 * (a + 1)],
                        ident[0 : H + 1, 0 : H + 1],
                    )
                    recip = posmall.tile([128, 1], f32, tag="recip")
                    nc.vector.reciprocal(recip[:], o_ps[:, H : H + 1])
                    o_sb = posmall.tile([128, H], f32, tag="o")
                    nc.vector.tensor_scalar(
                        o_sb[:], o_ps[:, 0:H], recip[:], None, op0=ALU.mult
                    )
                    r0 = QBLK * qb + 128 * a
                    nc.sync.dma_start(out_d[r0 : r0 + 128, :], o_sb[:])

    nc.compile()
    return nc


def _get_compiled():
    with _compile_lock:
        if "ncs" not in _compiled:
            _compiled["ncs"] = {h: _build_variant(h) for h in (0, 1)}
        return _compiled["ncs"]


def _make_runner(nc, devices):
    """Mirror of bass2jax.run_bass_via_pjrt's multi-core branch, but on an
    explicit device subset so two graphs can run concurrently."""
    import jax
    import numpy as np
    from jax.sharding import Mesh, PartitionSpec
    from jax.experimental.shard_map import shard_map

    import concourse.mybir as mybir
    from concourse import bass2jax

    bass2jax.install_neuronx_cc_hook()

    partition_name = nc.partition_id_tensor.name if nc.partition_id_tensor else None
    in_names, out_names, out_avals, zero_outs = [], [], [], []
    for alloc in nc.m.functions[0].allocations:
        if not isinstance(alloc, mybir.MemoryLocationSet):
            continue
        name = alloc.memorylocations[0].name
        if alloc.kind == "ExternalInput":
            if name != partition_name:
                in_names.append(name)
        elif alloc.kind == "ExternalOutput":
            out_names.append(name)
            shape = tuple(alloc.tensor_shape)
            dtype = mybir.dt.np(alloc.dtype)
            out_avals.append(jax.core.ShapedArray(shape, dtype))
            zero_outs.append(np.zeros(shape, dtype))
    n_params = len(in_names)
    n_outs = len(out_avals)
    all_in_names = list(in_names) + list(out_names)
    if partition_name is not None:
        all_in_names.append(partition_name)

    def _body(*args):
        operands = list(args)
        if partition_name is not None:
            operands.append(bass2jax.partition_id_tensor())
        outs = bass2jax._bass_exec_p.bind(
            *operands,
            out_avals=tuple(out_avals),
            in_names=tuple(all_in_names),
            out_names=tuple(out_names),
            lowering_input_output_aliases=(),
            sim_require_finite=True,
            sim_require_nnan=True,
            nc=nc,
        )
        return tuple(outs)

    n_cores = len(devices)
    mesh = Mesh(np.asarray(devices), ("core",))
    in_specs = (PartitionSpec("core"),) * (n_params + n_outs)
    out_specs = (PartitionSpec("core"),) * n_outs
    donate = tuple(range(n_params, n_params + n_outs))
    sharded = jax.jit(
        shard_map(
            _body, mesh=mesh, in_specs=in_specs, out_specs=out_specs, check_rep=False
        ),
        donate_argnums=donate,
        keep_unused=True,
    )

    def run(in_maps):
        assert len(in_maps) == n_cores
        concat_in = [
            np.concatenate([np.asarray(m[name]) for m in in_maps], axis=0)
            for name in in_names
        ]
        concat_zeros = [
            np.zeros((n_cores * z.shape[0], *z.shape[1:]), z.dtype) for z in zero_outs
        ]
        out_arrs = sharded(*concat_in, *concat_zeros)
        return out_arrs, out_names, out_avals

    return run


_runners = {}


def _get_runners():
    import jax

    if "r" not in _runners:
        ncs = _get_compiled()
        devs = jax.devices()
        _runners["r"] = {
            0: _make_runner(ncs[0], devs[0:4]),
            1: _make_runner(ncs[1], devs[4:8]),
        }
    return _runners["r"]


def kernel(x, Wk, Wq, Wv):
    x = np.ascontiguousarray(np.asarray(x, dtype=np.float32))
    Wk = np.ascontiguousarray(np.asarray(Wk, dtype=np.float32))
    Wq = np.ascontiguousarray(np.asarray(Wq, dtype=np.float32))
    Wv = np.ascontiguousarray(np.asarray(Wv, dtype=np.float32))
    runners = _get_runners()

    in_maps = {
        h: [{"x": x[b], "Wk": Wk, "Wq": Wq, "Wv": Wv} for b in range(B)] for h in (0, 1)
    }
    # dispatch both graphs; jax dispatch is async so they overlap on disjoint devices
    pending = {h: runners[h](in_maps[h]) for h in (0, 1)}

    out = np.empty((B, T, H), dtype=np.float32)
    for h in (0, 1):
        out_arrs, out_names, out_avals = pending[h]
        oi = out_names.index("out")
        per_core = np.asarray(out_arrs[oi]).reshape(B, *out_avals[oi].shape)
        for b in range(B):
            for j, blk in enumerate(HALF_BLOCKS[h]):
                out[b, QBLK * blk : QBLK * (blk + 1), :] = per_core[b][
                    QBLK * j : QBLK * (j + 1), :
                ]
    return out


# revision 10
# speedup vs baseline: 2.0797x; 2.0797x over previous
"""Trainium2 Bass kernel: single-head causal attention.

Reference computation (per batch b):
    K = x @ Wk; Q = x @ Wq; V = x @ Wv          (x: [T, C], W*: [C, H])
    S = Q @ K^T * C**-0.5, causal-masked
    out = softmax(S) @ V                        ([T, H])

Shapes hardcoded: B=4, T=4096, C=1024, H=64, f32.

Sharding: 8 cores = 4 batches x 2 causal-balanced query-halves.
  half 0 -> query blocks {0,1,6,7} of 512 rows, half 1 -> {2,3,4,5}.
Both halves have equal attention work (72 tk-tiles of 128) but different
instruction streams, so we compile TWO 4-core SPMD graphs (one per half)
and dispatch them concurrently on devices [0:4] and [4:8].

Per-core dataflow (all in one NeuronCore):
  1. Stream x in 512-row chunks; PE-transpose to xT [c,t] layout.
  2. Project: KT/VT[64,4096], QT[64,2048] = W.T @ xT  (PSUM accum over c).
  3. PE-transpose VT -> V [t, 65] with an appended ones column (row-sum trick).
  4. Per query block: ST[tk,tq] = K Q^T tiles, diag-masked, exp via ACT
     (scale 1/32 folded in), PV accumulated into OT[65, 512] PSUM,
     row 64 = softmax denominator. Transpose OT, normalize, DMA out.
"""

import os
import threading

import numpy as np

B, T, C, H = 4, 4096, 1024, 64
SCALE = float(C) ** -0.5  # 1/32
MASKVAL = -1.0e5  # pre-scale additive mask; exp(-1e5/32) == 0
QBLK = 512  # query block rows
HALF_BLOCKS = {0: (0, 1, 6, 7), 1: (2, 3, 4, 5)}
N_CORES_PER_GRAPH = 4

# matmul input dtype: f32 = 4 cyc/row, bf16 = 1 cyc/row (f32r crashes TRN2 NRT)
MM_DT_NAME = os.environ.get("BASS_MM_DT", "f32")

_compiled = {}
_compile_lock = threading.Lock()


def _build_variant(half: int):
    import concourse.bass as bass
    import concourse.mybir as mybir
    from concourse import bacc, tile
    from concourse._compat import get_trn_type
    from concourse.masks import make_identity

    f32 = mybir.dt.float32
    mm_dt = {
        "f32": mybir.dt.float32,
        "f32r": mybir.dt.float32r,
        "bf16": mybir.dt.bfloat16,
    }[MM_DT_NAME]
    AF = mybir.ActivationFunctionType
    ALU = mybir.AluOpType

    def mm(out, lhsT, rhs, **kw):
        nc.tensor.matmul(out, lhsT, rhs, **kw)

    nc = bacc.Bacc(
        get_trn_type() or "TRN2",
        target_bir_lowering=False,
        debug=False,
        num_devices=N_CORES_PER_GRAPH,
    )

    x_d = nc.dram_tensor("x", [T, C], f32, kind="ExternalInput").ap()
    wk_d = nc.dram_tensor("Wk", [C, H], f32, kind="ExternalInput").ap()
    wq_d = nc.dram_tensor("Wq", [C, H], f32, kind="ExternalInput").ap()
    wv_d = nc.dram_tensor("Wv", [C, H], f32, kind="ExternalInput").ap()
    out_d = nc.dram_tensor("out", [2048, H], f32, kind="ExternalOutput").ap()

    own = HALF_BLOCKS[half]
    n_chunks = T // QBLK  # 8 chunks of 512 rows
    n_ctiles = C // 128  # 8 contraction tiles

    with tile.TileContext(nc) as tc:
        with (
            tc.tile_pool(name="const", bufs=1) as pconst,
            tc.tile_pool(name="persist", bufs=1) as ppersist,
            tc.tile_pool(name="xin", bufs=2) as pxin,
            tc.tile_pool(name="xt", bufs=2) as pxt,
            tc.tile_pool(name="pt", bufs=3) as ppt,
            tc.tile_pool(name="otsb", bufs=2) as potsb,
            tc.tile_pool(name="osmall", bufs=3) as posmall,
            tc.tile_pool(name="ps_big", bufs=3, space="PSUM") as pbig,
            tc.tile_pool(name="ps_proj", bufs=2, space="PSUM") as pproj,
            tc.tile_pool(name="ps_acc", bufs=2, space="PSUM") as pacc,
            tc.tile_pool(name="ps_small", bufs=1, space="PSUM") as psmall,
        ):
            # ---- constants ----
            ident = pconst.tile([128, 128], f32, tag="ident")
            make_identity(nc, ident[:])
            # big_mask[p, j] = MASKVAL if p > j - 512 else 0   (j in [0,640))
            # diag tile with offset d uses columns [512-d, 640)
            big_mask = pconst.tile([128, 640], f32, tag="mask")
            nc.gpsimd.memset(big_mask[:], 0.0)
            nc.gpsimd.affine_select(
                out=big_mask[:],
                in_=big_mask[:],
                compare_op=ALU.is_ge,
                fill=MASKVAL,
                base=-512,
                channel_multiplier=-1,
                pattern=[[1, 640]],
            )

            # ---- weights: [128, 8, 64] (c_in-partition, c_tile, h) ----
            w_sb = {}
            for name, wd in (("k", wk_d), ("q", wq_d), ("v", wv_d)):
                w = pconst.tile([128, n_ctiles, H], mm_dt, tag=f"w{name}")
                if mm_dt is f32:
                    for cb in range(n_ctiles):
                        nc.sync.dma_start(
                            w[:, cb, :], wd[128 * cb : 128 * (cb + 1), :]
                        )
                else:
                    wstg = pconst.tile([128, n_ctiles, H], f32, tag=f"wstg{name}")
                    for cb in range(n_ctiles):
                        nc.sync.dma_start(
                            wstg[:, cb, :], wd[128 * cb : 128 * (cb + 1), :]
                        )
                    nc.vector.tensor_copy(w[:], wstg[:])
                w_sb[name] = w

            # ---- persistent projections ----
            kt_all = ppersist.tile([H, T], mm_dt, tag="kt")
            vt_all = ppersist.tile([H, T], f32, tag="vt")
            qt_all = ppersist.tile([H, 2048], mm_dt, tag="qt")
            v_sb = ppersist.tile([128, T // 128, H + 1], mm_dt, tag="v")
            if mm_dt is f32:
                nc.vector.memset(v_sb[:, :, H : H + 1], 1.0)
            else:
                ones_f32 = pconst.tile([128, T // 128], f32, tag="ones")
                nc.vector.memset(ones_f32[:], 1.0)
                nc.vector.tensor_copy(v_sb[:, :, H : H + 1], ones_f32[:, :])

            # ---- projection: stream x, transpose, project ----
            own_set = set(own)
            qi_of_chunk = {blk: i for i, blk in enumerate(own)}
            for ci in range(n_chunks):
                t0 = QBLK * ci
                x_chunk = pxin.tile([128, 4, C], f32, tag="xc")
                nc.sync.dma_start(
                    x_chunk[:],
                    x_d[t0 : t0 + QBLK, :].rearrange("(a p) c -> p a c", p=128),
                )
                xt_sb = pxt.tile([128, n_ctiles, QBLK], mm_dt, tag="xt")
                for cb in range(n_ctiles):
                    xt_ps = pbig.tile([128, QBLK], f32, tag="big")
                    for a in range(4):
                        nc.tensor.transpose(
                            xt_ps[:, 128 * a : 128 * (a + 1)],
                            x_chunk[:, a, 128 * cb : 128 * (cb + 1)],
                            ident[:],
                        )
                    nc.vector.tensor_copy(xt_sb[:, cb, :], xt_ps[:])

                # K and V share the PE array: col-groups (0,0) and (0,64)
                # stream the same xT tile once; psum rows 0-63 = K^T,
                # rows 64-127 = V^T.
                kv_ps = pproj.tile([128, QBLK], f32, tag="proj")
                for cb in range(n_ctiles):
                    nc.tensor.matmul(
                        kv_ps[0:H, :],
                        w_sb["k"][:, cb, :],
                        xt_sb[:, cb, :],
                        start=(cb == 0),
                        stop=(cb == n_ctiles - 1),
                        tile_position=(0, 0),
                    )
                    nc.tensor.matmul(
                        kv_ps[64 : 64 + H, :],
                        w_sb["v"][:, cb, :],
                        xt_sb[:, cb, :],
                        start=(cb == 0),
                        stop=(cb == n_ctiles - 1),
                        tile_position=(0, 64),
                    )
                nc.scalar.copy(kt_all[:, t0 : t0 + QBLK], kv_ps[0:H, :])
                nc.scalar.copy(vt_all[:, t0 : t0 + QBLK], kv_ps[64 : 64 + H, :])
                if ci in own_set:
                    q0 = QBLK * qi_of_chunk[ci]
                    prj = pproj.tile([H, QBLK], f32, tag="proj")
                    for cb in range(n_ctiles):
                        mm(
                            prj[:],
                            w_sb["q"][:, cb, :],
                            xt_sb[:, cb, :],
                            start=(cb == 0),
                            stop=(cb == n_ctiles - 1),
                        )
                    nc.scalar.copy(qt_all[:, q0 : q0 + QBLK], prj[:])

            # ---- V: [64, T] -> [t, 65] blocks (append ones col) ----
            for tt in range(T // 128):
                vps = psmall.tile([128, H], f32, tag="small")
                nc.tensor.transpose(
                    vps[:], vt_all[:, 128 * tt : 128 * (tt + 1)], ident[0:H, 0:H]
                )
                nc.scalar.copy(v_sb[:, tt, 0:H], vps[:])

            # ---- attention: process own blocks in pairs; the two ST
            # matmuls (contraction 64) share the PE array via row-groups
            # (0,0) and (64,0) and run concurrently ----
            def st_matmul(st, tk, qb, row):
                mm_kw = {"start": True, "stop": True, "tile_position": (row, 0)}
                nc.tensor.matmul(
                    st[:],
                    kt_all[:, 128 * tk : 128 * (tk + 1)],
                    qt_all[:, QBLK * qb : QBLK * (qb + 1)],
                    **mm_kw,
                )

            def mask_exp_pv(st, tk, qb, g_q, n_tk, ot):
                d = 128 * tk - g_q
                if d >= 0:
                    nc.vector.tensor_tensor(
                        st[:, 0 : d + 128],
                        st[:, 0 : d + 128],
                        big_mask[:, 512 - d : 640],
                        op=ALU.add,
                    )
                pt_sb = ppt.tile([128, QBLK], mm_dt, tag="pt")
                nc.scalar.activation(pt_sb[:], st[:], AF.Exp, scale=SCALE)
                mm(
                    ot[:],
                    v_sb[:, tk, :],
                    pt_sb[:],
                    start=(tk == 0),
                    stop=(tk == n_tk - 1),
                )

            def epilogue(qb, ot_ps):
                ot_sb = potsb.tile([H + 1, QBLK], f32, tag="ot")
                nc.scalar.copy(ot_sb[:], ot_ps[:])
                for a in range(4):
                    o_ps = psmall.tile([128, H + 1], f32, tag="small")
                    nc.tensor.transpose(
                        o_ps[:],
                        ot_sb[:, 128 * a : 128 * (a + 1)],
                        ident[0 : H + 1, 0 : H + 1],
                    )
                    recip = posmall.tile([128, 1], f32, tag="recip")
                    nc.vector.reciprocal(recip[:], o_ps[:, H : H + 1])
                    o_sb = posmall.tile([128, H], f32, tag="o")
                    nc.vector.tensor_scalar(
                        o_sb[:], o_ps[:, 0:H], recip[:], None, op0=ALU.mult
                    )
                    r0 = QBLK * qb + 128 * a
                    nc.sync.dma_start(out_d[r0 : r0 + 128, :], o_sb[:])

            for pair in range(2):
                qbA, qbB = 2 * pair, 2 * pair + 1
                blkA, blkB = own[qbA], own[qbB]
                gA, gB = QBLK * blkA, QBLK * blkB
                nA, nB = (gA + QBLK) // 128, (gB + QBLK) // 128
                otA = pacc.tile([H + 1, QBLK], f32, tag="acc")
                otB = pacc.tile([H + 1, QBLK], f32, tag="acc")
                for tk in range(nB):
                    stA = stB = None
                    if tk < nA:
                        stA = pbig.tile([128, QBLK], f32, tag="big")
                        st_matmul(stA, tk, qbA, 0)
                    stB = pbig.tile([128, QBLK], f32, tag="big")
                    st_matmul(stB, tk, qbB, 64)
                    if stA is not None:
                        mask_exp_pv(stA, tk, qbA, gA, nA, otA)
                    mask_exp_pv(stB, tk, qbB, gB, nB, otB)
                epilogue(qbA, otA)
                epilogue(qbB, otB)

    nc.compile()
    return nc


def _get_compiled():
    with _compile_lock:
        if "ncs" not in _compiled:
            _compiled["ncs"] = {h: _build_variant(h) for h in (0, 1)}
        return _compiled["ncs"]


def _make_runner(nc, devices):
    """Mirror of bass2jax.run_bass_via_pjrt's multi-core branch, but on an
    explicit device subset so two graphs can run concurrently."""
    import jax
    import numpy as np
    from jax.sharding import Mesh, PartitionSpec
    from jax.experimental.shard_map import shard_map

    import concourse.mybir as mybir
    from concourse import bass2jax

    bass2jax.install_neuronx_cc_hook()

    partition_name = nc.partition_id_tensor.name if nc.partition_id_tensor else None
    in_names, out_names, out_avals, zero_outs = [], [], [], []
    for alloc in nc.m.functions[0].allocations:
        if not isinstance(alloc, mybir.MemoryLocationSet):
            continue
        name = alloc.memorylocations[0].name
        if alloc.kind == "ExternalInput":
            if name != partition_name:
                in_names.append(name)
        elif alloc.kind == "ExternalOutput":
            out_names.append(name)
            shape = tuple(alloc.tensor_shape)
            dtype = mybir.dt.np(alloc.dtype)
            out_avals.append(jax.core.ShapedArray(shape, dtype))
            zero_outs.append(np.zeros(shape, dtype))
    n_params = len(in_names)
    n_outs = len(out_avals)
    all_in_names = list(in_names) + list(out_names)
    if partition_name is not None:
        all_in_names.append(partition_name)

    def _body(*args):
        operands = list(args)
        if partition_name is not None:
            operands.append(bass2jax.partition_id_tensor())
        outs = bass2jax._bass_exec_p.bind(
            *operands,
            out_avals=tuple(out_avals),
            in_names=tuple(all_in_names),
            out_names=tuple(out_names),
            lowering_input_output_aliases=(),
            sim_require_finite=True,
            sim_require_nnan=True,
            nc=nc,
        )
        return tuple(outs)

    n_cores = len(devices)
    mesh = Mesh(np.asarray(devices), ("core",))
    in_specs = (PartitionSpec("core"),) * (n_params + n_outs)
    out_specs = (PartitionSpec("core"),) * n_outs
    donate = tuple(range(n_params, n_params + n_outs))
    sharded = jax.jit(
        shard_map(
            _body, mesh=mesh, in_specs=in_specs, out_specs=out_specs, check_rep=False
        ),
        donate_argnums=donate,
        keep_unused=True,
    )

    def run(in_maps):
        assert len(in_maps) == n_cores
        concat_in = [
            np.concatenate([np.asarray(m[name]) for m in in_maps], axis=0)
            for name in in_names
        ]
        concat_zeros = [
            np.zeros((n_cores * z.shape[0], *z.shape[1:]), z.dtype) for z in zero_outs
        ]
        out_arrs = sharded(*concat_in, *concat_zeros)
        return out_arrs, out_names, out_avals

    return run


_runners = {}


def _get_runners():
    import jax

    if "r" not in _runners:
        ncs = _get_compiled()
        devs = jax.devices()
        _runners["r"] = {
            0: _make_runner(ncs[0], devs[0:4]),
            1: _make_runner(ncs[1], devs[4:8]),
        }
    return _runners["r"]


def kernel(x, Wk, Wq, Wv):
    x = np.ascontiguousarray(np.asarray(x, dtype=np.float32))
    Wk = np.ascontiguousarray(np.asarray(Wk, dtype=np.float32))
    Wq = np.ascontiguousarray(np.asarray(Wq, dtype=np.float32))
    Wv = np.ascontiguousarray(np.asarray(Wv, dtype=np.float32))
    runners = _get_runners()

    in_maps = {
        h: [{"x": x[b], "Wk": Wk, "Wq": Wq, "Wv": Wv} for b in range(B)] for h in (0, 1)
    }
    # dispatch both graphs; jax dispatch is async so they overlap on disjoint devices
    pending = {h: runners[h](in_maps[h]) for h in (0, 1)}

    out = np.empty((B, T, H), dtype=np.float32)
    for h in (0, 1):
        out_arrs, out_names, out_avals = pending[h]
        oi = out_names.index("out")
        per_core = np.asarray(out_arrs[oi]).reshape(B, *out_avals[oi].shape)
        for b in range(B):
            for j, blk in enumerate(HALF_BLOCKS[h]):
                out[b, QBLK * blk : QBLK * (blk + 1), :] = per_core[b][
                    QBLK * j : QBLK * (j + 1), :
                ]
    return out
